# revision 1
# baseline (speedup 1.0000x reference)
"""Trainium2 Bass kernel for nn_BaoCypherNet (tree-conv GNN).

Data-parallel over 8 NeuronCores: each core processes 256 trees.

Per tree, per layer l (C_l -> O_l, kernel 3, stride 3 over gathered triples):
  z[o, m] = sum_k sum_c W_k[o, c] * X[c, idx[3m+k]] + b[o]   (m = 1..127)
  x_cat = [0 | z]  -> layernorm over (O, 128) -> leaky_relu (layers 1, 2)

On-chip pipeline per group of G=4 trees:
  - GpSimd ap_gather pulls 384 columns (3 k-blocks x 128, k-major, last
    column of each block is a pad) from the [C, 128] activation tile.
  - PE: bias matmul (K=1, start=True zeroes psum) + 3 conv matmuls (f32r
    full-rate, or bf16 for layer 2's channel-pair layout) accumulate
    z in PSUM [O, G, 128] (col c = node c+1; col 127 garbage).
  - LN mean: DVE 3D-reduce -> ones-matmul (reduce across partitions AND
    broadcast to all 128 partitions in one op) -> scale by -1/K.
    The LN *scale* is deferred: leaky(s*x) = s*leaky(x) and the next
    layernorm is scale-invariant (up to the 1e-5 eps, relative error
    ~1e-5), so only layer 3's scale is ever materialized.
  - shift matmul (K=1, rhs stride-0) adds -mean into PSUM; ACT applies
    Prelu (leaky) while moving PSUM -> next layer's activation tile.
  - Layer-2 activations are stored as bf16 channel pairs (c, c+128)
    packed in 4-byte units so one gather moves all 256 channels.
  - Pooling: max over PSUM cols, max(x,0) handles the node-0 column;
    tiny MLP (W4 -> leaky -> W5) finishes on PE+DVE.
"""

import ml_dtypes
import numpy as np

import bass_rust as _bass_rust
import concourse.bass as bass
import concourse.mybir as mybir
from concourse import library_config
from concourse.bass_utils import run_bass_kernel_spmd
from concourse.library_overlay import lower_extended_insts
from concourse.tile import TileContext

F32 = mybir.dt.float32
F32R = mybir.dt.float32r
BF16 = mybir.dt.bfloat16
I16 = mybir.dt.int16

N_CORES = 8
B = 2048
BC = B // N_CORES  # trees per core
N = 128            # nodes (incl. zero-pad node 0)
M = N - 1          # conv output positions
G = 4              # trees per group
NGROUPS = BC // G
K1 = 256 * 128     # LN element counts per tree
K2 = 128 * 128
K3 = 64 * 128

_ALPHA = 0.01


def _ap(t_ap, extra_dims, offset_delta=0):
    """Build an AP on the same tensor with given free dims appended to the
    partition dim of `t_ap` (a full-tile AP)."""
    return bass.AP(
        tensor=t_ap.tensor,
        offset=t_ap.offset + offset_delta,
        ap=[t_ap.ap[0]] + list(extra_dims),
    )


def build_nc():
    nc = bass.Bass()

    trees_in = nc.dram_tensor("treesP", [128, BC, 128], F32, kind="ExternalInput")
    idx_in = nc.dram_tensor("idxP", [128, BC, 24], I16, kind="ExternalInput")
    w1_in = nc.dram_tensor("w1t", [128, 3, 2, 128], F32R, kind="ExternalInput")
    w2_in = nc.dram_tensor("w2t", [128, 3, 2, 128], BF16, kind="ExternalInput")
    w3_in = nc.dram_tensor("w3t", [128, 3, 64], F32R, kind="ExternalInput")
    brow_in = nc.dram_tensor("brows", [4, 128], F32R, kind="ExternalInput")
    ones_in = nc.dram_tensor("onesrow", [1, 128], F32R, kind="ExternalInput")
    ones128_in = nc.dram_tensor("ones128", [128, 128], F32, kind="ExternalInput")
    mlp_in = nc.dram_tensor("mlp_rhs", [66, 32], F32, kind="ExternalInput")
    w5_in = nc.dram_tensor("w5rep", [128, 32], F32, kind="ExternalInput")
    b5_in = nc.dram_tensor("b5rep", [128, 1], F32, kind="ExternalInput")
    out_dram = nc.dram_tensor("out", [BC, 1], F32, kind="ExternalOutput")

    nc.gpsimd.load_library(library_config.ap_gather)

    gather_insts = []

    with TileContext(nc) as tc:
        with (
            tc.tile_pool(name="const", bufs=1) as cp,
            tc.tile_pool(name="sb", bufs=3) as sb,
            tc.tile_pool(name="psz1", bufs=2, space="PSUM") as psz1,
            tc.tile_pool(name="pszB", bufs=2, space="PSUM") as pszB,
            tc.tile_pool(name="pss", bufs=2, space="PSUM") as pss,
        ):
            # ---- constants ----
            w1t = cp.tile([128, 3, 2, 128], F32R, tag="w1t")
            nc.sync.dma_start(out=w1t[:], in_=w1_in[:])
            w2t = cp.tile([128, 3, 2, 128], BF16, tag="w2t")
            nc.sync.dma_start(out=w2t[:], in_=w2_in[:])
            w3t = cp.tile([128, 3, 64], F32R, tag="w3t")
            nc.sync.dma_start(out=w3t[:], in_=w3_in[:])
            brow_tiles = []
            for i in range(4):
                bt_i = cp.tile([1, 128], F32R, tag=f"brow{i}")
                nc.sync.dma_start(out=bt_i[:], in_=brow_in[i:i + 1, :])
                brow_tiles.append(bt_i)
            onesrow = cp.tile([1, 128], F32R, tag="onesrow")
            nc.sync.dma_start(out=onesrow[:], in_=ones_in[:])
            ones128 = cp.tile([128, 128], F32, tag="ones128")
            nc.sync.dma_start(out=ones128[:], in_=ones128_in[:])
            mlp_rhs = cp.tile([66, 32], F32, tag="mlp_rhs")
            nc.sync.dma_start(out=mlp_rhs[:], in_=mlp_in[:])
            w5rep = cp.tile([128, 32], F32, tag="w5rep")
            nc.sync.dma_start(out=w5rep[:], in_=w5_in[:])
            b5rep = cp.tile([128, 1], F32, tag="b5rep")
            nc.sync.dma_start(out=b5rep[:], in_=b5_in[:])

            ones_rep = _ap(onesrow[:], [[0, G], [1, 128]])

            def gather(gtile, xtile_f32_view, itile):
                for t in range(G):
                    gather_insts.append(nc.gpsimd.ap_gather(
                        gtile[:, t, :], xtile_f32_view[:, t, :], itile[:, t, :],
                        channels=128, num_elems=128, d=1, num_idxs=384,
                    ).ins)

            for g in range(NGROUPS):
                t0 = g * G
                # ---- inputs for this group ----
                x1 = sb.tile([128, G, 128], F32, tag="x1")
                nc.sync.dma_start(out=x1[:], in_=trees_in[:, t0:t0 + G, :])
                idxt = sb.tile([128, G, 24], I16, tag="idxt")
                nc.sync.dma_start(out=idxt[:], in_=idx_in[:, t0:t0 + G, :])

                # ---- layer 1: C=128 f32 -> O=256 (2 chunks) ----
                g1 = sb.tile([128, G, 384], F32, tag="g1")
                gather(g1, x1, idxt)
                z1 = []
                for oc in range(2):
                    zt = psz1.tile([128, G, 128], F32, tag="z1")
                    z_all = _ap(zt[:], [[128, G], [1, 128]])
                    nc.tensor.matmul(
                        z_all, brow_tiles[oc][:], ones_rep,
                        start=True, stop=False)
                    for k in range(3):
                        rhs = _ap(g1[:], [[384, G], [1, 128]], k * 128)
                        nc.tensor.matmul(
                            z_all, w1t[:, k, oc, :], rhs.bitcast(F32R),
                            start=False, stop=(k == 2))
                    z1.append(zt)

                # LN1 mean
                s1 = sb.tile([128, 2, G], F32, tag="s1")
                for oc in range(2):
                    nc.vector.tensor_reduce(
                        s1[:, oc, :], _ap(z1[oc][:], [[128, G], [1, 127]]),
                        axis=mybir.AxisListType.X, op=mybir.AluOpType.add)
                ps1 = pss.tile([128, G], F32, tag="pss")
                nc.tensor.matmul(ps1[:], ones128[:], s1[:, 0, :],
                                 start=True, stop=False)
                nc.tensor.matmul(ps1[:], ones128[:], s1[:, 1, :],
                                 start=False, stop=True)
                nm1 = sb.tile([128, G], F32R, tag="nm1")
                with nc.allow_low_precision(reason="f32r is full width"):
                    nc.vector.tensor_scalar(
                        nm1[:], ps1[:], -1.0 / K1, None, mybir.AluOpType.mult)
                nm1f = nm1[:].bitcast(F32)
                nm1_rep = _ap(nm1[0:1, :], [[1, G], [0, 128]])
                for oc in range(2):
                    z_all = _ap(z1[oc][:], [[128, G], [1, 128]])
                    nc.tensor.matmul(z_all, onesrow[:], nm1_rep,
                                     start=False, stop=True,
                                     skip_group_check=True)

                # X2: bf16 channel pairs (c, c+128); node0 = leaky(-m1)
                x2 = sb.tile([128, G, 128, 2], BF16, tag="x2")
                t1a = sb.tile([128, G], F32, tag="t1a")
                nc.vector.tensor_scalar(
                    t1a[:], nm1f, _ALPHA, None, mybir.AluOpType.mult)
                nc.vector.tensor_tensor(
                    x2[:, :, 0, :],
                    _ap(nm1f, [[1, G], [0, 2]]),
                    _ap(t1a[:], [[1, G], [0, 2]]),
                    mybir.AluOpType.max)
                for oc in range(2):
                    nc.scalar.activation(
                        x2[:, :, 1:128, oc],
                        _ap(z1[oc][:], [[128, G], [1, 127]]),
                        mybir.ActivationFunctionType.Prelu,
                        bias=0.0, scale=1.0, alpha=_ALPHA)

                # ---- layer 2: C=256 bf16-pairs -> O=128 ----
                g2 = sb.tile([128, G, 384], F32, tag="g2")
                gather(g2, x2[:].bitcast(F32), idxt)
                z2 = pszB.tile([128, G, 128], F32, tag="z2")
                z2_all = _ap(z2[:], [[128, G], [1, 128]])
                nc.tensor.matmul(z2_all, brow_tiles[2][:], ones_rep,
                                 start=True, stop=False)
                g2b = g2[:].bitcast(BF16)
                for k in range(3):
                    for j in range(2):
                        rhs = bass.AP(
                            tensor=g2b.tensor,
                            offset=g2b.offset + k * 256 + j,
                            ap=[g2b.ap[0], [768, G], [2, 128]])
                        nc.tensor.matmul(
                            z2_all, w2t[:, k, j, :], rhs,
                            start=False, stop=(k == 2 and j == 1))

                # LN2 mean
                s2 = sb.tile([128, G], F32, tag="s2")
                nc.vector.tensor_reduce(
                    s2[:], _ap(z2[:], [[128, G], [1, 127]]),
                    axis=mybir.AxisListType.X, op=mybir.AluOpType.add)
                ps2 = pss.tile([128, G], F32, tag="pss")
                nc.tensor.matmul(ps2[:], ones128[:], s2[:], start=True, stop=True)
                nm2 = sb.tile([128, G], F32R, tag="nm2")
                with nc.allow_low_precision(reason="f32r is full width"):
                    nc.vector.tensor_scalar(
                        nm2[:], ps2[:], -1.0 / K2, None, mybir.AluOpType.mult)
                nm2f = nm2[:].bitcast(F32)
                nm2_rep = _ap(nm2[0:1, :], [[1, G], [0, 128]])
                nc.tensor.matmul(z2_all, onesrow[:], nm2_rep,
                                 start=False, stop=True, skip_group_check=True)

                x3 = sb.tile([128, G, 128], F32, tag="x3")
                t1b = sb.tile([128, G], F32, tag="t1b")
                nc.vector.tensor_scalar(
                    t1b[:], nm2f, _ALPHA, None, mybir.AluOpType.mult)
                nc.vector.tensor_tensor(
                    x3[:, :, 0], nm2f, t1b[:], mybir.AluOpType.max)
                nc.scalar.activation(
                    x3[:, :, 1:128], _ap(z2[:], [[128, G], [1, 127]]),
                    mybir.ActivationFunctionType.Prelu,
                    bias=0.0, scale=1.0, alpha=_ALPHA)

                # ---- layer 3: C=128 f32 -> O=64, LN (no leaky), max-pool ----
                g3 = sb.tile([128, G, 384], F32, tag="g3")
                gather(g3, x3, idxt)
                z3 = pszB.tile([64, G, 128], F32, tag="z3")
                z3_all = _ap(z3[:], [[128, G], [1, 128]])
                nc.tensor.matmul(z3_all, brow_tiles[3][:, 0:64], ones_rep,
                                 start=True, stop=False)
                for k in range(3):
                    rhs = _ap(g3[:], [[384, G], [1, 128]], k * 128)
                    nc.tensor.matmul(z3_all, w3t[:, k, :], rhs.bitcast(F32R),
                                     start=False, stop=(k == 2))

                # LN3 stats (mean and variance)
                z3v = _ap(z3[:], [[128, G], [1, 127]])
                s3 = sb.tile([64, 2, G], F32, tag="s3")
                nc.vector.tensor_reduce(
                    s3[:, 0, :], z3v, axis=mybir.AxisListType.X,
                    op=mybir.AluOpType.add)
                sq = sb.tile([64, G, 127], F32, tag="sq")
                nc.scalar.activation(
                    sq[:], z3v, mybir.ActivationFunctionType.Square,
                    bias=0.0, scale=1.0)
                nc.vector.tensor_reduce(
                    s3[:, 1, :], sq[:], axis=mybir.AxisListType.X,
                    op=mybir.AluOpType.add)
                ps3 = pss.tile([128, 2, G], F32, tag="pss")
                nc.tensor.matmul(
                    ps3[:], ones128[0:64, :],
                    bass.AP(tensor=s3.tensor, offset=s3[:].offset,
                            ap=[s3[:].ap[0], [1, 2 * G]]),
                    start=True, stop=True)
                # mean3 = S/K3; nm3 = -mean3; var = SS/(K3-1) - K3/(K3-1)*mean^2
                mean3 = sb.tile([128, G], F32, tag="mean3")
                nc.vector.tensor_scalar(
                    mean3[:], ps3[:, 0, :], 1.0 / K3, None, mybir.AluOpType.mult)
                nm3 = sb.tile([128, G], F32, tag="nm3")
                nc.vector.tensor_scalar(
                    nm3[:], mean3[:], -1.0, None, mybir.AluOpType.mult)
                m3sq = sb.tile([128, G], F32, tag="m3sq")
                nc.vector.tensor_tensor(
                    m3sq[:], mean3[:], mean3[:], mybir.AluOpType.mult)
                var3 = sb.tile([128, G], F32, tag="var3")
                nc.vector.tensor_scalar(
                    var3[:], m3sq[:], -float(K3) / (K3 - 1), None,
                    mybir.AluOpType.mult)
                ssn = sb.tile([128, G], F32, tag="ssn")
                nc.vector.tensor_scalar(
                    ssn[:], ps3[:, 1, :], 1.0 / (K3 - 1), None,
                    mybir.AluOpType.mult)
                nc.vector.tensor_tensor(
                    var3[:], var3[:], ssn[:], mybir.AluOpType.add)
                std3 = sb.tile([128, G], F32, tag="std3")
                nc.scalar.activation(
                    std3[:], var3[:], mybir.ActivationFunctionType.Sqrt,
                    bias=0.0, scale=1.0)
                nc.vector.tensor_scalar(
                    std3[:], std3[:], 1e-5, None, mybir.AluOpType.add)
                sinv3 = sb.tile([128, G], F32, tag="sinv3")
                nc.vector.reciprocal(sinv3[:], std3[:])

                # pooled = s3 * (max(max_m z3, 0) - mean3); node0 via Relu
                pr = sb.tile([64, G], F32, tag="pr")
                nc.vector.tensor_reduce(
                    pr[:], z3v, axis=mybir.AxisListType.X,
                    op=mybir.AluOpType.max)
                paug = sb.tile([66, G], F32, tag="paug")
                nc.vector.memset(paug[64:66, :], 1.0)
                r1 = sb.tile([64, G], F32, tag="r1")
                nc.vector.tensor_scalar(
                    r1[:], pr[:], 0.0, None, mybir.AluOpType.max)
                r2 = sb.tile([64, G], F32, tag="r2")
                nc.vector.tensor_tensor(
                    r2[:], r1[:], nm3[0:64, :], mybir.AluOpType.add)
                nc.vector.tensor_tensor(
                    paug[0:64, :], r2[:], sinv3[0:64, :], mybir.AluOpType.mult)

                # h = leaky(W4 @ pooled + b4); out = h @ W5.T + b5
                ph = pss.tile([G, 32], F32, tag="pss")
                nc.tensor.matmul(ph[:], paug[:, :], mlp_rhs[0:66, :],
                                 start=True, stop=True)
                h = sb.tile([G, 32], F32, tag="h")
                nc.scalar.activation(
                    h[:], ph[:], mybir.ActivationFunctionType.Prelu,
                    bias=0.0, scale=1.0, alpha=_ALPHA)
                prod = sb.tile([G, 32], F32, tag="prod")
                nc.vector.tensor_tensor(
                    prod[:], h[:], w5rep[0:G, :], mybir.AluOpType.mult)
                ov = sb.tile([G, 1], F32, tag="ov")
                nc.vector.tensor_reduce(
                    ov[:], prod[:], axis=mybir.AxisListType.X,
                    op=mybir.AluOpType.add)
                nc.vector.tensor_scalar(
                    ov[:], ov[:], b5rep[0:G, :], None, mybir.AluOpType.add)
                nc.sync.dma_start(out=out_dram[t0:t0 + G, :], in_=ov[:])

    _bass_rust.generate_event_semaphores(nc)
    lower_extended_insts(nc)
    for gi in gather_insts:
        gi.outs[0].dtype = F32R
    nc.finalize()
    return nc


_NC_CACHE = None


def _get_nc():
    global _NC_CACHE
    if _NC_CACHE is None:
        _NC_CACHE = build_nc()
    return _NC_CACHE


def _prep_idx(indexes: np.ndarray) -> np.ndarray:
    """indexes [B, 381] -> wrapped int16 [B, 128, 24] (k-major padded)."""
    b = indexes.shape[0]
    idxk = np.zeros((b, 3, 128), np.int16)
    tri = indexes.reshape(b, 127, 3).astype(np.int16)
    idxk[:, :, :127] = tri.transpose(0, 2, 1)
    flat = idxk.reshape(b, 384)
    wrapped = flat.reshape(b, 24, 16).transpose(0, 2, 1)  # [b, 16, 24]
    return np.tile(wrapped, (1, 8, 1))  # [b, 128, 24]


def kernel(trees, W1, b1, W2, b2, W3, b3, W4, b4, W5, b5, indexes):
    trees = np.asarray(trees, dtype=np.float32)
    indexes = np.asarray(indexes).astype(np.int64)
    W1 = np.asarray(W1, dtype=np.float32)
    W2 = np.asarray(W2, dtype=np.float32)
    W3 = np.asarray(W3, dtype=np.float32)
    W4 = np.asarray(W4, dtype=np.float32)
    W5 = np.asarray(W5, dtype=np.float32)
    b1 = np.asarray(b1, dtype=np.float32)
    b2 = np.asarray(b2, dtype=np.float32)
    b3 = np.asarray(b3, dtype=np.float32)
    b4 = np.asarray(b4, dtype=np.float32)
    b5 = np.asarray(b5, dtype=np.float32)

    nc = _get_nc()

    # replicated weight prep
    # w1t[c, k, oc, o] = W1[oc*128+o, c, k]
    w1t = np.ascontiguousarray(
        W1.reshape(2, 128, 128, 3).transpose(2, 3, 0, 1))
    # w2t[p, k, j, o] = W2[o, j*128+p, k]
    w2t = np.ascontiguousarray(
        W2.reshape(128, 2, 128, 3).transpose(2, 3, 1, 0))
    w2t_bf = w2t.astype(ml_dtypes.bfloat16)
    # w3t[c, k, o] = W3[o, c, k]
    w3t = np.ascontiguousarray(W3.transpose(1, 2, 0))
    brows = np.zeros((4, 128), np.float32)
    brows[0] = b1[:128]
    brows[1] = b1[128:]
    brows[2] = b2
    brows[3, :64] = b3
    onesrow = np.ones((1, 128), np.float32)
    ones128 = np.ones((128, 128), np.float32)
    mlp_rhs = np.zeros((66, 32), np.float32)
    mlp_rhs[:64] = W4.T
    mlp_rhs[64] = b4 * 0.5
    mlp_rhs[65] = b4 * 0.5
    w5rep = np.tile(W5.reshape(1, 32), (128, 1)).astype(np.float32)
    b5rep = np.full((128, 1), b5[0], np.float32)

    idx_wrapped = _prep_idx(indexes)  # [B, 128, 24] int16

    in_maps = []
    for c in range(N_CORES):
        lo, hi = c * BC, (c + 1) * BC
        treesP = np.ascontiguousarray(trees[lo:hi].transpose(1, 0, 2))
        idxP = np.ascontiguousarray(idx_wrapped[lo:hi].transpose(1, 0, 2))
        in_maps.append({
            "treesP": treesP,
            "idxP": idxP,
            "w1t": w1t, "w2t": w2t_bf, "w3t": w3t, "brows": brows,
            "onesrow": onesrow, "ones128": ones128,
            "mlp_rhs": mlp_rhs, "w5rep": w5rep, "b5rep": b5rep,
        })

    global _LAST_IN_MAPS
    _LAST_IN_MAPS = in_maps
    res = run_bass_kernel_spmd(nc, in_maps, list(range(N_CORES)))
    out = np.concatenate([res.results[c]["out"] for c in range(N_CORES)], axis=0)
    return out.astype(np.float32)


_LAST_IN_MAPS = None



# revision 10
# speedup vs baseline: 3.3225x; 3.3225x over previous
"""Trainium2 Bass kernel for nn_BaoCypherNet (tree-conv GNN).

Data-parallel over 8 NeuronCores: each core processes 256 trees.

Per tree, per layer l (C_l -> O_l, kernel 3, stride 3 over gathered triples):
  z[o, m] = sum_k sum_c W_k[o, c] * X[c, idx[3m+k]] + b[o]   (m = 1..127)
  x_cat = [0 | z]  -> layernorm over (O, 128) -> leaky_relu (layers 1, 2)

The gather runs ON THE PE as a one-hot matmul (GPSIMD ap_gather costs
~10.7us per tree-layer; the PE does the same selection in ~160ns):
  - One-hot P[n, j] = (n == idx[j]) is host-built (bf16, pad columns
    zeroed) and DMA'd; j is k-major (j = k*128 + m).
  - gather: g[c, j] = sum_n xT[n, c] * P[n, j] -- stationary = node-major
    activation tile xT, moving = P.  Layer 1's xT comes from a
    host-transposed DMA; layers 2/3 get xT via PE transpose-mode matmuls.
  - g is copied PSUM->SBUF into k-major layout; the convs are batched
    over a group of G=4 trees (stationary weights, moving gathered data,
    N=512), all bf16.
  - Biases ride the scalar-engine Prelu (per-partition bias AP), not
    matmuls.  LN mean-subtract is ONE K=128 matmul per PSUM chunk: ones
    stationary, the (-1/K)-scaled per-partition sums broadcast as moving
    operand (partition-reduce + broadcast + subtract fused).  Since the
    pad columns of P are zero, PSUM column 127 is exactly -mean after
    the shift, which is the node-0 value leaky(-mean) after Prelu.
  - LN *scale* is deferred for layers 1/2 (leaky(s*x) = s*leaky(x), next
    LN is scale-invariant up to eps); only layer 3 materializes stats,
    with b3 folded in via activation-bias on the Square and pooled path.
  - Pooling: max over PSUM cols, max(x+b3,0) handles the node-0 column;
    tiny MLP (W4 -> leaky -> W5) finishes on PE+DVE.
"""

import ml_dtypes
import numpy as np

import bass_rust as _bass_rust
import concourse.bass as bass
import concourse.mybir as mybir
from concourse.bass_utils import run_bass_kernel_spmd
from concourse.tile import TileContext

F32 = mybir.dt.float32
BF16 = mybir.dt.bfloat16

N_CORES = 8
B = 2048
BC = B // N_CORES  # trees per core
N = 128            # nodes (incl. zero-pad node 0)
M = N - 1          # conv output positions
G = 4              # trees per group
NGROUPS = BC // G
K1 = 256 * 128     # LN element counts per tree
K2 = 128 * 128
K3 = 64 * 128

_ALPHA = 0.01


def _ap(t_ap, extra_dims, offset_delta=0):
    """Build an AP on the same tensor with given free dims appended to the
    partition dim of `t_ap` (a full-tile AP)."""
    return bass.AP(
        tensor=t_ap.tensor,
        offset=t_ap.offset + offset_delta,
        ap=[t_ap.ap[0]] + list(extra_dims),
    )


def build_nc():
    nc = bass.Bass()

    treesT_in = nc.dram_tensor("treesT", [128, BC, 128], BF16, kind="ExternalInput")
    p_in = nc.dram_tensor("ponehot", [128, BC, 384], BF16, kind="ExternalInput")
    w1_in = nc.dram_tensor("w1t", [128, 3, 2, 128], BF16, kind="ExternalInput")
    w2_in = nc.dram_tensor("w2t", [128, 3, 2, 128], BF16, kind="ExternalInput")
    w3_in = nc.dram_tensor("w3t", [128, 3, 64], BF16, kind="ExternalInput")
    bc_in = nc.dram_tensor("bcols", [128, 8], F32, kind="ExternalInput")
    ones128b_in = nc.dram_tensor("ones128b", [128, 128], BF16, kind="ExternalInput")
    ones128f_in = nc.dram_tensor("ones128f", [128, 128], F32, kind="ExternalInput")
    ident_in = nc.dram_tensor("identb", [128, 128], BF16, kind="ExternalInput")
    mlp_in = nc.dram_tensor("mlp_rhs", [66, 32], F32, kind="ExternalInput")
    w5_in = nc.dram_tensor("w5rep", [128, 32], F32, kind="ExternalInput")
    b5_in = nc.dram_tensor("b5rep", [128, 1], F32, kind="ExternalInput")
    out_dram = nc.dram_tensor("out", [BC, 1], F32, kind="ExternalOutput")

    with TileContext(nc) as tc:
        with (
            tc.tile_pool(name="const", bufs=1) as cp,
            tc.tile_pool(name="sb", bufs=3) as sb,
            tc.tile_pool(name="psz1", bufs=2, space="PSUM") as psz1,
            tc.tile_pool(name="pszB", bufs=1, space="PSUM") as pszB,
            tc.tile_pool(name="psg", bufs=4, space="PSUM") as psg,
        ):
            # ---- constants ----
            w1t = cp.tile([128, 3, 2, 128], BF16, tag="w1t")
            nc.sync.dma_start(out=w1t[:], in_=w1_in[:])
            w2t = cp.tile([128, 3, 2, 128], BF16, tag="w2t")
            nc.sync.dma_start(out=w2t[:], in_=w2_in[:])
            w3t = cp.tile([128, 3, 64], BF16, tag="w3t")
            nc.sync.dma_start(out=w3t[:], in_=w3_in[:])
            # bcols: [b1_lo, b1_hi, b2, b3pad, 127*b3pad, mb1, mb2, 0]
            bcols = cp.tile([128, 8], F32, tag="bcols")
            nc.sync.dma_start(out=bcols[:], in_=bc_in[:])
            ones128b = cp.tile([128, 128], BF16, tag="ones128b")
            nc.sync.dma_start(out=ones128b[:], in_=ones128b_in[:])
            ones128f = cp.tile([128, 128], F32, tag="ones128f")
            nc.sync.dma_start(out=ones128f[:], in_=ones128f_in[:])
            identb = cp.tile([128, 128], BF16, tag="identb")
            nc.sync.dma_start(out=identb[:], in_=ident_in[:])
            mlp_rhs = cp.tile([66, 32], F32, tag="mlp_rhs")
            nc.sync.dma_start(out=mlp_rhs[:], in_=mlp_in[:])
            w5rep = cp.tile([128, 32], F32, tag="w5rep")
            nc.sync.dma_start(out=w5rep[:], in_=w5_in[:])
            b5rep = cp.tile([128, 1], F32, tag="b5rep")
            nc.sync.dma_start(out=b5rep[:], in_=b5_in[:])

            for g in range(NGROUPS):
                t0 = g * G
                # ---- inputs for this group ----
                x1T = sb.tile([128, G, 128], BF16, tag="x1T")
                nc.sync.dma_start(out=x1T[:], in_=treesT_in[:, t0:t0 + G, :])
                P = sb.tile([128, G, 384], BF16, tag="P")
                nc.sync.dma_start(out=P[:], in_=p_in[:, t0:t0 + G, :])

                # ---- layer 1 gathers: g1[c, j] = x1[c, idx[j]] ----
                g1 = sb.tile([128, 3, G, 128], BF16, tag="g1")
                for t in range(G):
                    gp = psg.tile([128, 384], F32, tag="g", name=f"g1p{t}")
                    nc.tensor.matmul(gp[:], x1T[:, t, :], P[:, t, :],
                                     start=True, stop=True)
                    nc.scalar.activation(
                        _ap(g1[:], [[G * 128, 3], [1, 128]], t * 128),
                        gp[:], mybir.ActivationFunctionType.Copy,
                        bias=0.0, scale=1.0)

                # ---- layer 1 convs: C=128 -> O=256 (2 chunks) ----
                z1 = []
                for oc in range(2):
                    zt = psz1.tile([128, G, 128], F32, tag="z1")
                    z_all = _ap(zt[:], [[128, G], [1, 128]])
                    for k in range(3):
                        rhs = _ap(g1[:], [[128, G], [1, 128]], k * G * 128)
                        nc.tensor.matmul(z_all, w1t[:, k, oc, :], rhs,
                                         start=(k == 0), stop=(k == 2))
                    z1.append(zt)

                # LN1 mean: per-partition sums -> fused -mean via ones MM
                s1 = sb.tile([128, 2, G], F32, tag="s1")
                for oc in range(2):
                    nc.vector.tensor_reduce(
                        s1[:, oc, :], _ap(z1[oc][:], [[128, G], [1, 127]]),
                        axis=mybir.AxisListType.X, op=mybir.AluOpType.add)
                s1sum = sb.tile([128, G], F32, tag="s1sum")
                nc.vector.tensor_tensor(
                    s1sum[:], s1[:, 0, :], s1[:, 1, :], mybir.AluOpType.add)
                s1n = sb.tile([128, G], BF16, tag="s1n")
                with nc.allow_low_precision(reason="mean shift in bf16"):
                    nc.vector.tensor_scalar(
                        s1n[:], s1sum[:], -1.0 / K1, bcols[:, 5:6],
                        mybir.AluOpType.mult, mybir.AluOpType.add)
                s1n_rep = _ap(s1n[:], [[1, G], [0, 128]])
                for oc in range(2):
                    z_all = _ap(z1[oc][:], [[128, G], [1, 128]])
                    nc.tensor.matmul(z_all, ones128b[:], s1n_rep,
                                     start=False, stop=True,
                                     skip_group_check=True)

                # X2: bf16 channel pairs (c, c+128); nodes 1..127 get
                # leaky(z + b1 - mean) (bias via ACT); node 0 comes from
                # PSUM col 127, which is exactly -mean (pad cols of P = 0).
                x2 = sb.tile([128, G, 128, 2], BF16, tag="x2")
                for oc in range(2):
                    nc.scalar.activation(
                        x2[:, :, 1:128, oc],
                        _ap(z1[oc][:], [[128, G], [1, 127]]),
                        mybir.ActivationFunctionType.Prelu,
                        bias=bcols[:, oc:oc + 1], scale=1.0, alpha=_ALPHA)
                    nc.scalar.activation(
                        x2[:, :, 0, oc],
                        _ap(z1[oc][:], [[128, G]], 127),
                        mybir.ActivationFunctionType.Prelu,
                        bias=0.0, scale=1.0, alpha=_ALPHA)

                # ---- layer 2: transpose x2 chunks, gather, conv ----
                g2 = sb.tile([128, 3, G, 128, 2], BF16, tag="g2")
                for t in range(G):
                    for j in range(2):
                        xtp = psg.tile([128, 128], BF16, tag="g",
                                       name=f"xtp{t}{j}")
                        nc.tensor.transpose(
                            xtp[:],
                            bass.AP(tensor=x2.tensor,
                                    offset=x2[:].offset + t * 256 + j,
                                    ap=[x2[:].ap[0], [2, 128]]),
                            identb[:])
                        x2Ts = sb.tile([128, 128], BF16, tag="x2Ts")
                        nc.vector.tensor_copy(x2Ts[:], xtp[:])
                        gp = psg.tile([128, 384], F32, tag="g",
                                      name=f"g2p{t}{j}")
                        nc.tensor.matmul(gp[:], x2Ts[:], P[:, t, :],
                                         start=True, stop=True)
                        nc.scalar.activation(
                            _ap(g2[:], [[G * 256, 3], [2, 128]], t * 256 + j),
                            gp[:], mybir.ActivationFunctionType.Copy,
                            bias=0.0, scale=1.0)

                z2 = pszB.tile([128, G, 128], F32, tag="z2")
                z2_all = _ap(z2[:], [[128, G], [1, 128]])
                for k in range(3):
                    for j in range(2):
                        rhs = _ap(g2[:], [[256, G], [2, 128]],
                                  k * G * 256 + j)
                        nc.tensor.matmul(z2_all, w2t[:, k, j, :], rhs,
                                         start=(k == 0 and j == 0),
                                         stop=(k == 2 and j == 1))

                # LN2 mean
                s2 = sb.tile([128, G], F32, tag="s2")
                nc.vector.tensor_reduce(
                    s2[:], _ap(z2[:], [[128, G], [1, 127]]),
                    axis=mybir.AxisListType.X, op=mybir.AluOpType.add)
                s2n = sb.tile([128, G], BF16, tag="s2n")
                with nc.allow_low_precision(reason="mean shift in bf16"):
                    nc.vector.tensor_scalar(
                        s2n[:], s2[:], -1.0 / K2, bcols[:, 6:7],
                        mybir.AluOpType.mult, mybir.AluOpType.add)
                nc.tensor.matmul(z2_all, ones128b[:],
                                 _ap(s2n[:], [[1, G], [0, 128]]),
                                 start=False, stop=True, skip_group_check=True)

                x3 = sb.tile([128, G, 128], BF16, tag="x3")
                nc.scalar.activation(
                    x3[:, :, 1:128], _ap(z2[:], [[128, G], [1, 127]]),
                    mybir.ActivationFunctionType.Prelu,
                    bias=bcols[:, 2:3], scale=1.0, alpha=_ALPHA)
                nc.scalar.activation(
                    x3[:, :, 0], _ap(z2[:], [[128, G]], 127),
                    mybir.ActivationFunctionType.Prelu,
                    bias=0.0, scale=1.0, alpha=_ALPHA)

                # ---- layer 3: transpose x3, gather, conv, LN, max-pool ----
                g3 = sb.tile([128, 3, G, 128], BF16, tag="g3")
                for t in range(G):
                    xtp = psg.tile([128, 128], BF16, tag="g", name=f"xtp3{t}")
                    nc.tensor.transpose(xtp[:], x3[:, t, :], identb[:])
                    x3Ts = sb.tile([128, 128], BF16, tag="x3Ts")
                    nc.vector.tensor_copy(x3Ts[:], xtp[:])
                    gp = psg.tile([128, 384], F32, tag="g", name=f"g3p{t}")
                    nc.tensor.matmul(gp[:], x3Ts[:], P[:, t, :],
                                     start=True, stop=True)
                    with nc.allow_low_precision(reason="gather copy to bf16"):
                        nc.vector.tensor_copy(
                            _ap(g3[:], [[G * 128, 3], [1, 128]], t * 128),
                            gp[:])

                z3 = pszB.tile([64, G, 128], F32, tag="z3")
                z3_all = _ap(z3[:], [[128, G], [1, 128]])
                for k in range(3):
                    rhs = _ap(g3[:], [[128, G], [1, 128]], k * G * 128)
                    nc.tensor.matmul(z3_all, w3t[:, k, :], rhs,
                                     start=(k == 0), stop=(k == 2))

                # LN3 stats on z3+b3 (b3 via ACT bias / post-corrections)
                z3v = _ap(z3[:], [[128, G], [1, 127]])
                s3 = sb.tile([64, 2, G], F32, tag="s3")
                nc.vector.tensor_reduce(
                    s3[:, 0, :], z3v, axis=mybir.AxisListType.X,
                    op=mybir.AluOpType.add)
                nc.vector.tensor_scalar(
                    s3[:, 0, :], s3[:, 0, :], bcols[0:64, 4:5], None,
                    mybir.AluOpType.add)
                sq = sb.tile([64, G, 127], F32, tag="sq")
                nc.scalar.activation(
                    sq[:], z3v, mybir.ActivationFunctionType.Square,
                    bias=bcols[0:64, 3:4], scale=1.0)
                nc.vector.tensor_reduce(
                    s3[:, 1, :], sq[:], axis=mybir.AxisListType.X,
                    op=mybir.AluOpType.add)
                ps3 = psg.tile([128, 2, G], F32, tag="g", name="ps3")
                nc.tensor.matmul(
                    ps3[:], ones128f[0:64, :],
                    bass.AP(tensor=s3.tensor, offset=s3[:].offset,
                            ap=[s3[:].ap[0], [1, 2 * G]]),
                    start=True, stop=True)
                # mean3 = S/K3; nm3 = -mean3; var = SS/(K3-1) - K3/(K3-1)*mean^2
                mean3 = sb.tile([128, G], F32, tag="mean3")
                nc.vector.tensor_scalar(
                    mean3[:], ps3[:, 0, :], 1.0 / K3, None, mybir.AluOpType.mult)
                nm3 = sb.tile([128, G], F32, tag="nm3")
                nc.vector.tensor_scalar(
                    nm3[:], mean3[:], -1.0, None, mybir.AluOpType.mult)
                m3sq = sb.tile([128, G], F32, tag="m3sq")
                nc.vector.tensor_tensor(
                    m3sq[:], mean3[:], mean3[:], mybir.AluOpType.mult)
                var3 = sb.tile([128, G], F32, tag="var3")
                nc.vector.tensor_scalar(
                    var3[:], m3sq[:], -float(K3) / (K3 - 1), None,
                    mybir.AluOpType.mult)
                ssn = sb.tile([128, G], F32, tag="ssn")
                nc.vector.tensor_scalar(
                    ssn[:], ps3[:, 1, :], 1.0 / (K3 - 1), None,
                    mybir.AluOpType.mult)
                nc.vector.tensor_tensor(
                    var3[:], var3[:], ssn[:], mybir.AluOpType.add)
                std3 = sb.tile([128, G], F32, tag="std3")
                nc.scalar.activation(
                    std3[:], var3[:], mybir.ActivationFunctionType.Sqrt,
                    bias=0.0, scale=1.0)
                nc.vector.tensor_scalar(
                    std3[:], std3[:], 1e-5, None, mybir.AluOpType.add)
                sinv3 = sb.tile([128, G], F32, tag="sinv3")
                nc.vector.reciprocal(sinv3[:], std3[:])

                # pooled = sinv3 * (max(max_m z3 + b3, 0) - mean3)
                pr = sb.tile([64, G], F32, tag="pr")
                nc.vector.tensor_reduce(
                    pr[:], z3v, axis=mybir.AxisListType.X,
                    op=mybir.AluOpType.max)
                paug = sb.tile([66, G], F32, tag="paug")
                nc.vector.memset(paug[64:66, :], 1.0)
                r1 = sb.tile([64, G], F32, tag="r1")
                nc.vector.tensor_scalar(
                    r1[:], pr[:], bcols[0:64, 3:4], 0.0,
                    mybir.AluOpType.add, mybir.AluOpType.max)
                r2 = sb.tile([64, G], F32, tag="r2")
                nc.vector.tensor_tensor(
                    r2[:], r1[:], nm3[0:64, :], mybir.AluOpType.add)
                nc.vector.tensor_tensor(
                    paug[0:64, :], r2[:], sinv3[0:64, :], mybir.AluOpType.mult)

                # h = leaky(W4 @ pooled + b4); out = h @ W5.T + b5
                ph = psg.tile([G, 32], F32, tag="g", name="ph")
                nc.tensor.matmul(ph[:], paug[:, :], mlp_rhs[0:66, :],
                                 start=True, stop=True)
                h = sb.tile([G, 32], F32, tag="h")
                nc.scalar.activation(
                    h[:], ph[:], mybir.ActivationFunctionType.Prelu,
                    bias=0.0, scale=1.0, alpha=_ALPHA)
                prod = sb.tile([G, 32], F32, tag="prod")
                nc.vector.tensor_tensor(
                    prod[:], h[:], w5rep[0:G, :], mybir.AluOpType.mult)
                ov = sb.tile([G, 1], F32, tag="ov")
                nc.vector.tensor_reduce(
                    ov[:], prod[:], axis=mybir.AxisListType.X,
                    op=mybir.AluOpType.add)
                nc.vector.tensor_scalar(
                    ov[:], ov[:], b5rep[0:G, :], None, mybir.AluOpType.add)
                nc.sync.dma_start(out=out_dram[t0:t0 + G, :], in_=ov[:])

    _bass_rust.generate_event_semaphores(nc)
    nc.finalize()
    return nc


_NC_CACHE = None


def _get_nc():
    global _NC_CACHE
    if _NC_CACHE is None:
        _NC_CACHE = build_nc()
    return _NC_CACHE


def _prep_idx_flat(indexes: np.ndarray) -> np.ndarray:
    """indexes [B, 381] -> k-major [B, 384] int32 with pads = -1."""
    b = indexes.shape[0]
    idxk = np.full((b, 3, 128), -1, np.int32)
    tri = indexes.reshape(b, 127, 3).astype(np.int32)
    idxk[:, :, :127] = tri.transpose(0, 2, 1)
    return idxk.reshape(b, 384)


def kernel(trees, W1, b1, W2, b2, W3, b3, W4, b4, W5, b5, indexes):
    trees = np.asarray(trees, dtype=np.float32)
    indexes = np.asarray(indexes).astype(np.int64)
    W1 = np.asarray(W1, dtype=np.float32)
    W2 = np.asarray(W2, dtype=np.float32)
    W3 = np.asarray(W3, dtype=np.float32)
    W4 = np.asarray(W4, dtype=np.float32)
    W5 = np.asarray(W5, dtype=np.float32)
    b1 = np.asarray(b1, dtype=np.float32)
    b2 = np.asarray(b2, dtype=np.float32)
    b3 = np.asarray(b3, dtype=np.float32)
    b4 = np.asarray(b4, dtype=np.float32)
    b5 = np.asarray(b5, dtype=np.float32)

    nc = _get_nc()

    bf = ml_dtypes.bfloat16
    # replicated weight prep
    # w1t[c, k, oc, o] = W1[oc*128+o, c, k]
    w1t = np.ascontiguousarray(
        W1.reshape(2, 128, 128, 3).transpose(2, 3, 0, 1)).astype(bf)
    # w2t[p, k, j, o] = W2[o, j*128+p, k]
    w2t = np.ascontiguousarray(
        W2.reshape(128, 2, 128, 3).transpose(2, 3, 1, 0)).astype(bf)
    # w3t[c, k, o] = W3[o, c, k]
    w3t = np.ascontiguousarray(W3.transpose(1, 2, 0)).astype(bf)
    # bias columns + scalar mean-bias corrections
    bcols = np.zeros((128, 8), np.float32)
    bcols[:, 0] = b1[:128]
    bcols[:, 1] = b1[128:]
    bcols[:, 2] = b2
    bcols[:64, 3] = b3
    bcols[:64, 4] = 127.0 * b3
    bcols[:, 5] = -127.0 * float(b1.sum()) / (K1 * 128)  # L1 shift mean-bias
    bcols[:, 6] = -127.0 * float(b2.sum()) / (K2 * 128)  # L2 shift mean-bias
    ones128b = np.ones((128, 128), bf)
    ones128f = np.ones((128, 128), np.float32)
    identb = np.eye(128, dtype=np.float32).astype(bf)
    mlp_rhs = np.zeros((66, 32), np.float32)
    mlp_rhs[:64] = W4.T
    mlp_rhs[64] = b4 * 0.5
    mlp_rhs[65] = b4 * 0.5
    w5rep = np.tile(W5.reshape(1, 32), (128, 1)).astype(np.float32)
    b5rep = np.full((128, 1), b5[0], np.float32)

    idxk = _prep_idx_flat(indexes)  # [B, 384] int32, pads -1
    # one-hot P[b, n, j] = (idxk[b, j] == n), pads give zero columns
    onehot = (idxk[:, None, :] == np.arange(128, dtype=np.int32)[None, :, None])
    onehot = onehot.astype(bf)  # [B, 128, 384]

    in_maps = []
    for c in range(N_CORES):
        lo, hi = c * BC, (c + 1) * BC
        treesT = np.ascontiguousarray(
            trees[lo:hi].transpose(2, 0, 1)).astype(bf)  # [n, t, c]
        ponehot = np.ascontiguousarray(
            onehot[lo:hi].transpose(1, 0, 2))  # [128, BC, 384]
        in_maps.append({
            "treesT": treesT,
            "ponehot": ponehot,
            "w1t": w1t, "w2t": w2t, "w3t": w3t, "bcols": bcols,
            "ones128b": ones128b, "ones128f": ones128f,
            "identb": identb,
            "mlp_rhs": mlp_rhs, "w5rep": w5rep, "b5rep": b5rep,
        })

    global _LAST_IN_MAPS
    _LAST_IN_MAPS = in_maps
    res = run_bass_kernel_spmd(nc, in_maps, list(range(N_CORES)))
    out = np.concatenate([res.results[c]["out"] for c in range(N_CORES)], axis=0)
    return out.astype(np.float32)


_LAST_IN_MAPS = None


# revision 15
# speedup vs baseline: 5.2799x; 1.5891x over previous
"""Trainium2 Bass kernel for nn_BaoCypherNet (tree-conv GNN).

Data-parallel over 8 NeuronCores: each core processes 256 trees.

Per tree, per layer l (C_l -> O_l, kernel 3, stride 3 over gathered triples):
  z[o, m] = sum_k sum_c W_k[o, c] * X[c, idx[3m+k]] + b[o]   (m = 1..127)
  x_cat = [0 | z]  -> layernorm over (O, 128) -> leaky_relu (layers 1, 2)

The gather runs ON THE PE as a one-hot matmul (GPSIMD ap_gather costs
~10.7us per tree-layer; the PE does the same selection in ~160ns):
  - One-hot P[n, j] = (n == idx[j]) is host-built (bf16, pad columns
    zeroed) and DMA'd; j is k-major (j = k*128 + m).
  - gather: g[c, j] = sum_n xT[n, c] * P[n, j] -- stationary = node-major
    activation tile xT, moving = P.  Layer 1's xT comes from a
    host-transposed DMA; layers 2/3 get xT via PE transpose-mode matmuls.
  - g is copied PSUM->SBUF into k-major layout; the convs are batched
    over a group of G=4 trees (stationary weights, moving gathered data,
    N=512), all bf16.
  - Biases ride the scalar-engine Prelu (per-partition bias AP), not
    matmuls.  LN mean-subtract is ONE K=128 matmul per PSUM chunk: ones
    stationary, the (-1/K)-scaled per-partition sums broadcast as moving
    operand (partition-reduce + broadcast + subtract fused).  Since the
    pad columns of P are zero, PSUM column 127 is exactly -mean after
    the shift, which is the node-0 value leaky(-mean) after Prelu.
  - LN *scale* is deferred for layers 1/2 (leaky(s*x) = s*leaky(x), next
    LN is scale-invariant up to eps); only layer 3 materializes stats,
    with b3 folded in via activation-bias on the Square and pooled path.
  - Pooling: max over PSUM cols, max(x+b3,0) handles the node-0 column;
    tiny MLP (W4 -> leaky -> W5) finishes on PE+DVE.
"""

import ml_dtypes
import numpy as np

import bass_rust as _bass_rust
import concourse.bass as bass
import concourse.mybir as mybir
from concourse.bass_utils import run_bass_kernel_spmd
from concourse.tile import TileContext

F32 = mybir.dt.float32
BF16 = mybir.dt.bfloat16

N_CORES = 8
B = 2048
BC = B // N_CORES  # trees per core
N = 128            # nodes (incl. zero-pad node 0)
M = N - 1          # conv output positions
G = 4              # trees per group
NGROUPS = BC // G
K1 = 256 * 128     # LN element counts per tree
K2 = 128 * 128
K3 = 64 * 128

_ALPHA = 0.01


def _ap(t_ap, extra_dims, offset_delta=0):
    """Build an AP on the same tensor with given free dims appended to the
    partition dim of `t_ap` (a full-tile AP)."""
    return bass.AP(
        tensor=t_ap.tensor,
        offset=t_ap.offset + offset_delta,
        ap=[t_ap.ap[0]] + list(extra_dims),
    )


def build_nc():
    nc = bass.Bass()

    treesT_in = nc.dram_tensor("treesT", [128, BC, 128], BF16, kind="ExternalInput")
    p_in = nc.dram_tensor("ponehot", [128, BC, 384], BF16, kind="ExternalInput")
    w1_in = nc.dram_tensor("w1t", [128, 3, 2, 128], BF16, kind="ExternalInput")
    w2_in = nc.dram_tensor("w2t", [128, 3, 2, 128], BF16, kind="ExternalInput")
    w3_in = nc.dram_tensor("w3t", [128, 3, 64], BF16, kind="ExternalInput")
    bc_in = nc.dram_tensor("bcols", [128, 8], F32, kind="ExternalInput")
    ones128b_in = nc.dram_tensor("ones128b", [128, 128], BF16, kind="ExternalInput")
    ones128f_in = nc.dram_tensor("ones128f", [128, 128], F32, kind="ExternalInput")
    ident_in = nc.dram_tensor("identb", [128, 128], BF16, kind="ExternalInput")
    mlp_in = nc.dram_tensor("mlp_rhs", [66, 32], F32, kind="ExternalInput")
    w5_in = nc.dram_tensor("w5rep", [128, 32], F32, kind="ExternalInput")
    b5_in = nc.dram_tensor("b5rep", [128, 1], F32, kind="ExternalInput")
    out_dram = nc.dram_tensor("out", [BC, 1], F32, kind="ExternalOutput")

    with TileContext(nc) as tc:
        with (
            tc.tile_pool(name="const", bufs=1) as cp,
            tc.tile_pool(name="sb", bufs=3) as sb,
            tc.tile_pool(name="psz1", bufs=2, space="PSUM") as psz1,
            tc.tile_pool(name="pszB", bufs=1, space="PSUM") as pszB,
            tc.tile_pool(name="psg", bufs=2, space="PSUM") as psg,
            tc.tile_pool(name="psx", bufs=1, space="PSUM") as psx,
            tc.tile_pool(name="pss", bufs=1, space="PSUM") as pss,
        ):
            # ---- constants ----
            w1t = cp.tile([128, 3, 2, 128], BF16, tag="w1t")
            nc.sync.dma_start(out=w1t[:], in_=w1_in[:])
            w2t = cp.tile([128, 3, 2, 128], BF16, tag="w2t")
            nc.sync.dma_start(out=w2t[:], in_=w2_in[:])
            w3t = cp.tile([128, 3, 64], BF16, tag="w3t")
            nc.sync.dma_start(out=w3t[:], in_=w3_in[:])
            # bcols: [b1_lo, b1_hi, b2, b3pad, 127*b3pad, mb1, mb2, 0]
            bcols = cp.tile([128, 8], F32, tag="bcols")
            nc.sync.dma_start(out=bcols[:], in_=bc_in[:])
            ones128b = cp.tile([128, 128], BF16, tag="ones128b")
            nc.sync.dma_start(out=ones128b[:], in_=ones128b_in[:])
            ones128f = cp.tile([128, 128], F32, tag="ones128f")
            nc.sync.dma_start(out=ones128f[:], in_=ones128f_in[:])
            identb = cp.tile([128, 128], BF16, tag="identb")
            nc.sync.dma_start(out=identb[:], in_=ident_in[:])
            mlp_rhs = cp.tile([66, 32], F32, tag="mlp_rhs")
            nc.sync.dma_start(out=mlp_rhs[:], in_=mlp_in[:])
            w5rep = cp.tile([128, 32], F32, tag="w5rep")
            nc.sync.dma_start(out=w5rep[:], in_=w5_in[:])
            b5rep = cp.tile([128, 1], F32, tag="b5rep")
            nc.sync.dma_start(out=b5rep[:], in_=b5_in[:])

            for g in range(NGROUPS):
                t0 = g * G
                # ---- inputs for this group ----
                x1T = sb.tile([128, G, 128], BF16, tag="x1T")
                nc.sync.dma_start(out=x1T[:], in_=treesT_in[:, t0:t0 + G, :])
                P = sb.tile([128, G, 384], BF16, tag="P")
                nc.sync.dma_start(out=P[:], in_=p_in[:, t0:t0 + G, :])

                # ---- layer 1 gathers: g1[c, j] = x1[c, idx[j]] ----
                g1 = sb.tile([128, 3, G, 128], BF16, tag="g1")
                for t in range(G):
                    gp = psg.tile([128, 384], F32, tag="g", name=f"g1p{t}")
                    nc.tensor.matmul(gp[:], x1T[:, t, :], P[:, t, :],
                                     start=True, stop=True)
                    nc.scalar.activation(
                        _ap(g1[:], [[G * 128, 3], [1, 128]], t * 128),
                        gp[:], mybir.ActivationFunctionType.Copy,
                        bias=0.0, scale=1.0)

                # ---- layer 1 convs: C=128 -> O=256 (2 chunks) ----
                z1 = []
                for oc in range(2):
                    zt = psz1.tile([128, G, 128], F32, tag="z1")
                    z_all = _ap(zt[:], [[128, G], [1, 128]])
                    for k in range(3):
                        rhs = _ap(g1[:], [[128, G], [1, 128]], k * G * 128)
                        nc.tensor.matmul(z_all, w1t[:, k, oc, :], rhs,
                                         start=(k == 0), stop=(k == 2))
                    z1.append(zt)

                # LN1 mean: per-partition sums -> fused -mean via ones MM
                s1 = sb.tile([128, 2, G], F32, tag="s1")
                for oc in range(2):
                    nc.vector.tensor_reduce(
                        s1[:, oc, :], _ap(z1[oc][:], [[128, G], [1, 127]]),
                        axis=mybir.AxisListType.X, op=mybir.AluOpType.add)
                s1sum = sb.tile([128, G], F32, tag="s1sum")
                nc.vector.tensor_tensor(
                    s1sum[:], s1[:, 0, :], s1[:, 1, :], mybir.AluOpType.add)
                s1n = sb.tile([128, G], BF16, tag="s1n")
                with nc.allow_low_precision(reason="mean shift in bf16"):
                    nc.vector.tensor_scalar(
                        s1n[:], s1sum[:], -1.0 / K1, bcols[:, 5:6],
                        mybir.AluOpType.mult, mybir.AluOpType.add)
                s1n_rep = _ap(s1n[:], [[1, G], [0, 128]])
                for oc in range(2):
                    z_all = _ap(z1[oc][:], [[128, G], [1, 128]])
                    nc.tensor.matmul(z_all, ones128b[:], s1n_rep,
                                     start=False, stop=True,
                                     skip_group_check=True)

                # X2: bf16 channel pairs (c, c+128); nodes 1..127 get
                # leaky(z + b1 - mean) (bias via ACT); node 0 comes from
                # PSUM col 127, which is exactly -mean (pad cols of P = 0).
                x2 = sb.tile([128, G, 128, 2], BF16, tag="x2")
                for oc in range(2):
                    nc.scalar.activation(
                        x2[:, :, 1:128, oc],
                        _ap(z1[oc][:], [[128, G], [1, 127]]),
                        mybir.ActivationFunctionType.Prelu,
                        bias=bcols[:, oc:oc + 1], scale=1.0, alpha=_ALPHA)
                    nc.scalar.activation(
                        x2[:, :, 0, oc],
                        _ap(z1[oc][:], [[128, G]], 127),
                        mybir.ActivationFunctionType.Prelu,
                        bias=0.0, scale=1.0, alpha=_ALPHA)

                # ---- layer 2: transpose x2 chunks, then gather, conv ----
                g2 = sb.tile([128, 3, G, 128, 2], BF16, tag="g2")
                x2T = sb.tile([128, 2 * G, 128], BF16, tag="x2T")
                for t in range(G):
                    for j in range(2):
                        xtp = psx.tile([128, 128], BF16, tag="xt",
                                       name=f"xtp{t}{j}")
                        nc.tensor.transpose(
                            xtp[:],
                            bass.AP(tensor=x2.tensor,
                                    offset=x2[:].offset + t * 256 + j,
                                    ap=[x2[:].ap[0], [2, 128]]),
                            identb[:])
                        nc.vector.tensor_copy(x2T[:, 2 * t + j, :], xtp[:])
                for t in range(G):
                    for j in range(2):
                        gp = psg.tile([128, 384], F32, tag="g",
                                      name=f"g2p{t}{j}")
                        nc.tensor.matmul(gp[:], x2T[:, 2 * t + j, :],
                                         P[:, t, :], start=True, stop=True)
                        nc.scalar.activation(
                            _ap(g2[:], [[G * 256, 3], [2, 128]], t * 256 + j),
                            gp[:], mybir.ActivationFunctionType.Copy,
                            bias=0.0, scale=1.0)

                z2 = pszB.tile([128, G, 128], F32, tag="z2")
                z2_all = _ap(z2[:], [[128, G], [1, 128]])
                for k in range(3):
                    for j in range(2):
                        rhs = _ap(g2[:], [[256, G], [2, 128]],
                                  k * G * 256 + j)
                        nc.tensor.matmul(z2_all, w2t[:, k, j, :], rhs,
                                         start=(k == 0 and j == 0),
                                         stop=(k == 2 and j == 1))

                # LN2 mean
                s2 = sb.tile([128, G], F32, tag="s2")
                nc.vector.tensor_reduce(
                    s2[:], _ap(z2[:], [[128, G], [1, 127]]),
                    axis=mybir.AxisListType.X, op=mybir.AluOpType.add)
                s2n = sb.tile([128, G], BF16, tag="s2n")
                with nc.allow_low_precision(reason="mean shift in bf16"):
                    nc.vector.tensor_scalar(
                        s2n[:], s2[:], -1.0 / K2, bcols[:, 6:7],
                        mybir.AluOpType.mult, mybir.AluOpType.add)
                nc.tensor.matmul(z2_all, ones128b[:],
                                 _ap(s2n[:], [[1, G], [0, 128]]),
                                 start=False, stop=True, skip_group_check=True)

                x3 = sb.tile([128, G, 128], BF16, tag="x3")
                nc.scalar.activation(
                    x3[:, :, 1:128], _ap(z2[:], [[128, G], [1, 127]]),
                    mybir.ActivationFunctionType.Prelu,
                    bias=bcols[:, 2:3], scale=1.0, alpha=_ALPHA)
                nc.scalar.activation(
                    x3[:, :, 0], _ap(z2[:], [[128, G]], 127),
                    mybir.ActivationFunctionType.Prelu,
                    bias=0.0, scale=1.0, alpha=_ALPHA)

                # ---- layer 3: transpose x3, then gather, conv, LN, pool ----
                g3 = sb.tile([128, 3, G, 128], BF16, tag="g3")
                x3T = sb.tile([128, G, 128], BF16, tag="x3T")
                for t in range(G):
                    xtp = psx.tile([128, 128], BF16, tag="xt", name=f"xtp3{t}")
                    nc.tensor.transpose(xtp[:], x3[:, t, :], identb[:])
                    nc.vector.tensor_copy(x3T[:, t, :], xtp[:])
                for t in range(G):
                    gp = psg.tile([128, 384], F32, tag="g", name=f"g3p{t}")
                    nc.tensor.matmul(gp[:], x3T[:, t, :], P[:, t, :],
                                     start=True, stop=True)
                    nc.scalar.activation(
                        _ap(g3[:], [[G * 128, 3], [1, 128]], t * 128),
                        gp[:], mybir.ActivationFunctionType.Copy,
                        bias=0.0, scale=1.0)

                z3 = pszB.tile([64, G, 128], F32, tag="z3")
                z3_all = _ap(z3[:], [[128, G], [1, 128]])
                for k in range(3):
                    rhs = _ap(g3[:], [[128, G], [1, 128]], k * G * 128)
                    nc.tensor.matmul(z3_all, w3t[:, k, :], rhs,
                                     start=(k == 0), stop=(k == 2))

                # LN3 stats on z3+b3 (b3 via ACT bias / post-corrections)
                z3v = _ap(z3[:], [[128, G], [1, 127]])
                s3 = sb.tile([64, 2, G], F32, tag="s3")
                nc.vector.tensor_reduce(
                    s3[:, 0, :], z3v, axis=mybir.AxisListType.X,
                    op=mybir.AluOpType.add)
                nc.vector.tensor_scalar(
                    s3[:, 0, :], s3[:, 0, :], bcols[0:64, 4:5], None,
                    mybir.AluOpType.add)
                sq = sb.tile([64, G, 127], F32, tag="sq")
                nc.scalar.activation(
                    sq[:], z3v, mybir.ActivationFunctionType.Square,
                    bias=bcols[0:64, 3:4], scale=1.0)
                nc.vector.tensor_reduce(
                    s3[:, 1, :], sq[:], axis=mybir.AxisListType.X,
                    op=mybir.AluOpType.add)
                ps3 = pss.tile([128, 2, G], F32, tag="pss", name="ps3")
                nc.tensor.matmul(
                    ps3[:], ones128f[0:64, :],
                    bass.AP(tensor=s3.tensor, offset=s3[:].offset,
                            ap=[s3[:].ap[0], [1, 2 * G]]),
                    start=True, stop=True)
                # mean3 = S/K3; nm3 = -mean3; var = SS/(K3-1) - K3/(K3-1)*mean^2
                mean3 = sb.tile([128, G], F32, tag="mean3")
                nc.vector.tensor_scalar(
                    mean3[:], ps3[:, 0, :], 1.0 / K3, None, mybir.AluOpType.mult)
                nm3 = sb.tile([128, G], F32, tag="nm3")
                nc.vector.tensor_scalar(
                    nm3[:], mean3[:], -1.0, None, mybir.AluOpType.mult)
                m3sq = sb.tile([128, G], F32, tag="m3sq")
                nc.vector.tensor_tensor(
                    m3sq[:], mean3[:], mean3[:], mybir.AluOpType.mult)
                var3 = sb.tile([128, G], F32, tag="var3")
                nc.vector.tensor_scalar(
                    var3[:], m3sq[:], -float(K3) / (K3 - 1), None,
                    mybir.AluOpType.mult)
                ssn = sb.tile([128, G], F32, tag="ssn")
                nc.vector.tensor_scalar(
                    ssn[:], ps3[:, 1, :], 1.0 / (K3 - 1), None,
                    mybir.AluOpType.mult)
                nc.vector.tensor_tensor(
                    var3[:], var3[:], ssn[:], mybir.AluOpType.add)
                std3 = sb.tile([128, G], F32, tag="std3")
                nc.scalar.activation(
                    std3[:], var3[:], mybir.ActivationFunctionType.Sqrt,
                    bias=0.0, scale=1.0)
                nc.vector.tensor_scalar(
                    std3[:], std3[:], 1e-5, None, mybir.AluOpType.add)
                sinv3 = sb.tile([128, G], F32, tag="sinv3")
                nc.vector.reciprocal(sinv3[:], std3[:])

                # pooled = sinv3 * (max(max_m z3 + b3, 0) - mean3)
                pr = sb.tile([64, G], F32, tag="pr")
                nc.vector.tensor_reduce(
                    pr[:], z3v, axis=mybir.AxisListType.X,
                    op=mybir.AluOpType.max)
                paug = sb.tile([66, G], F32, tag="paug")
                nc.vector.memset(paug[64:66, :], 1.0)
                r1 = sb.tile([64, G], F32, tag="r1")
                nc.vector.tensor_scalar(
                    r1[:], pr[:], bcols[0:64, 3:4], 0.0,
                    mybir.AluOpType.add, mybir.AluOpType.max)
                r2 = sb.tile([64, G], F32, tag="r2")
                nc.vector.tensor_tensor(
                    r2[:], r1[:], nm3[0:64, :], mybir.AluOpType.add)
                nc.vector.tensor_tensor(
                    paug[0:64, :], r2[:], sinv3[0:64, :], mybir.AluOpType.mult)

                # h = leaky(W4 @ pooled + b4); out = h @ W5.T + b5
                ph = pss.tile([G, 32], F32, tag="pss", name="ph")
                nc.tensor.matmul(ph[:], paug[:, :], mlp_rhs[0:66, :],
                                 start=True, stop=True)
                h = sb.tile([G, 32], F32, tag="h")
                nc.scalar.activation(
                    h[:], ph[:], mybir.ActivationFunctionType.Prelu,
                    bias=0.0, scale=1.0, alpha=_ALPHA)
                prod = sb.tile([G, 32], F32, tag="prod")
                nc.vector.tensor_tensor(
                    prod[:], h[:], w5rep[0:G, :], mybir.AluOpType.mult)
                ov = sb.tile([G, 1], F32, tag="ov")
                nc.vector.tensor_reduce(
                    ov[:], prod[:], axis=mybir.AxisListType.X,
                    op=mybir.AluOpType.add)
                nc.vector.tensor_scalar(
                    ov[:], ov[:], b5rep[0:G, :], None, mybir.AluOpType.add)
                nc.sync.dma_start(out=out_dram[t0:t0 + G, :], in_=ov[:])

    _bass_rust.generate_event_semaphores(nc)
    nc.finalize()
    return nc


_NC_CACHE = None


def _get_nc():
    global _NC_CACHE
    if _NC_CACHE is None:
        _NC_CACHE = build_nc()
    return _NC_CACHE


def _prep_idx_flat(indexes: np.ndarray) -> np.ndarray:
    """indexes [B, 381] -> k-major [B, 384] int32 with pads = -1."""
    b = indexes.shape[0]
    idxk = np.full((b, 3, 128), -1, np.int32)
    tri = indexes.reshape(b, 127, 3).astype(np.int32)
    idxk[:, :, :127] = tri.transpose(0, 2, 1)
    return idxk.reshape(b, 384)


def kernel(trees, W1, b1, W2, b2, W3, b3, W4, b4, W5, b5, indexes):
    trees = np.asarray(trees, dtype=np.float32)
    indexes = np.asarray(indexes).astype(np.int64)
    W1 = np.asarray(W1, dtype=np.float32)
    W2 = np.asarray(W2, dtype=np.float32)
    W3 = np.asarray(W3, dtype=np.float32)
    W4 = np.asarray(W4, dtype=np.float32)
    W5 = np.asarray(W5, dtype=np.float32)
    b1 = np.asarray(b1, dtype=np.float32)
    b2 = np.asarray(b2, dtype=np.float32)
    b3 = np.asarray(b3, dtype=np.float32)
    b4 = np.asarray(b4, dtype=np.float32)
    b5 = np.asarray(b5, dtype=np.float32)

    nc = _get_nc()

    bf = ml_dtypes.bfloat16
    # replicated weight prep
    # w1t[c, k, oc, o] = W1[oc*128+o, c, k]
    w1t = np.ascontiguousarray(
        W1.reshape(2, 128, 128, 3).transpose(2, 3, 0, 1)).astype(bf)
    # w2t[p, k, j, o] = W2[o, j*128+p, k]
    w2t = np.ascontiguousarray(
        W2.reshape(128, 2, 128, 3).transpose(2, 3, 1, 0)).astype(bf)
    # w3t[c, k, o] = W3[o, c, k]
    w3t = np.ascontiguousarray(W3.transpose(1, 2, 0)).astype(bf)
    # bias columns + scalar mean-bias corrections
    bcols = np.zeros((128, 8), np.float32)
    bcols[:, 0] = b1[:128]
    bcols[:, 1] = b1[128:]
    bcols[:, 2] = b2
    bcols[:64, 3] = b3
    bcols[:64, 4] = 127.0 * b3
    bcols[:, 5] = -127.0 * float(b1.sum()) / (K1 * 128)  # L1 shift mean-bias
    bcols[:, 6] = -127.0 * float(b2.sum()) / (K2 * 128)  # L2 shift mean-bias
    ones128b = np.ones((128, 128), bf)
    ones128f = np.ones((128, 128), np.float32)
    identb = np.eye(128, dtype=np.float32).astype(bf)
    mlp_rhs = np.zeros((66, 32), np.float32)
    mlp_rhs[:64] = W4.T
    mlp_rhs[64] = b4 * 0.5
    mlp_rhs[65] = b4 * 0.5
    w5rep = np.tile(W5.reshape(1, 32), (128, 1)).astype(np.float32)
    b5rep = np.full((128, 1), b5[0], np.float32)

    idxk = _prep_idx_flat(indexes)  # [B, 384] int32, pads -1
    # one-hot P[b, n, j] = (idxk[b, j] == n), pads give zero columns
    onehot = (idxk[:, None, :] == np.arange(128, dtype=np.int32)[None, :, None])
    onehot = onehot.astype(bf)  # [B, 128, 384]

    in_maps = []
    for c in range(N_CORES):
        lo, hi = c * BC, (c + 1) * BC
        treesT = np.ascontiguousarray(
            trees[lo:hi].transpose(2, 0, 1)).astype(bf)  # [n, t, c]
        ponehot = np.ascontiguousarray(
            onehot[lo:hi].transpose(1, 0, 2))  # [128, BC, 384]
        in_maps.append({
            "treesT": treesT,
            "ponehot": ponehot,
            "w1t": w1t, "w2t": w2t, "w3t": w3t, "bcols": bcols,
            "ones128b": ones128b, "ones128f": ones128f,
            "identb": identb,
            "mlp_rhs": mlp_rhs, "w5rep": w5rep, "b5rep": b5rep,
        })

    global _LAST_IN_MAPS
    _LAST_IN_MAPS = in_maps
    res = run_bass_kernel_spmd(nc, in_maps, list(range(N_CORES)))
    out = np.concatenate([res.results[c]["out"] for c in range(N_CORES)], axis=0)
    return out.astype(np.float32)


_LAST_IN_MAPS = None


# revision 19
# speedup vs baseline: 6.8276x; 1.2931x over previous
"""Trainium2 Bass kernel for nn_BaoCypherNet (tree-conv GNN).

Data-parallel over 8 NeuronCores: each core processes 256 trees.

Per tree, per layer l (C_l -> O_l, kernel 3, stride 3 over gathered triples):
  z[o, m] = sum_k sum_c W_k[o, c] * X[c, idx[3m+k]] + b[o]   (m = 1..127)
  x_cat = [0 | z]  -> layernorm over (O, 128) -> leaky_relu (layers 1, 2)

Node-major ("transposed") pipeline -- activations live as [node, channel]
tiles so the one-hot gather matmul needs NO transposes anywhere:
  - One-hot P[n, j] = (n == idx[j]) is host-built (bf16, pad columns
    zeroed) and DMA'd; j is k-major (j = k*128 + m).
  - gather: g[c, j] = sum_n x[n, c] * P[n, j] -- stationary = the
    node-major activation tile slice, moving = P.  PSUM -> SBUF copy.
  - convs (layers 1/2) run "reversed": stationary = gathered k-block
    g_k[c, m], moving = weights W_k[c, o]; output z^T[m, o] lands
    node-major, exactly the layout the next gather wants.  Layer 3 uses
    channel-major convs (stationary weights, N=512 over G=4 trees) since
    its output feeds the free-dim max-pool, not another gather.
  - bias = K=1 matmul (ones column [1,127] stationary) over node rows
    only; LN mean-subtract = one K=128 matmul (ones stationary, scaled
    per-row sums broadcast as moving operand).  P's pad columns are
    zero, so PSUM row 127 is exactly -mean after the shift: the node-0
    row of the next activation is leaky(-mean), written by one
    broadcast DVE copy.
  - LN *scale* is deferred for layers 1/2 (leaky(s*x) = s*leaky(x), next
    LN is scale-invariant up to eps); only layer 3 materializes stats,
    with b3 folded in via activation-bias on the Square and pooled path.
  - Pooling: max over PSUM cols, max(x+b3,0) handles the node-0 column;
    tiny MLP (W4 -> leaky -> W5) finishes on PE+DVE.
"""

import ml_dtypes
import numpy as np

import bass_rust as _bass_rust
import concourse.bass as bass
import concourse.mybir as mybir
from concourse.bass_utils import run_bass_kernel_spmd
from concourse.tile import TileContext

F32 = mybir.dt.float32
BF16 = mybir.dt.bfloat16

N_CORES = 8
B = 2048
BC = B // N_CORES  # trees per core
N = 128            # nodes (incl. zero-pad node 0)
M = N - 1          # conv output positions
G = 4              # trees per group
NGROUPS = BC // G
K1 = 256 * 128     # LN element counts per tree
K2 = 128 * 128
K3 = 64 * 128

_ALPHA = 0.01


def _ap(t_ap, extra_dims, offset_delta=0):
    """Build an AP on the same tensor with given free dims appended to the
    partition dim of `t_ap` (a full-tile AP)."""
    return bass.AP(
        tensor=t_ap.tensor,
        offset=t_ap.offset + offset_delta,
        ap=[t_ap.ap[0]] + list(extra_dims),
    )


def build_nc():
    nc = bass.Bass()

    treesT_in = nc.dram_tensor("treesT", [128, BC, 128], BF16, kind="ExternalInput")
    p_in = nc.dram_tensor("ponehot", [128, BC, 384], BF16, kind="ExternalInput")
    w1_in = nc.dram_tensor("w1r", [128, 3, 256], BF16, kind="ExternalInput")
    w2_in = nc.dram_tensor("w2t", [128, 3, 2, 128], BF16, kind="ExternalInput")
    w3_in = nc.dram_tensor("w3t", [128, 3, 64], BF16, kind="ExternalInput")
    br_in = nc.dram_tensor("biasrows", [1, 384], BF16, kind="ExternalInput")
    oc_in = nc.dram_tensor("onescol", [1, 128], BF16, kind="ExternalInput")
    bc_in = nc.dram_tensor("bcols", [128, 8], F32, kind="ExternalInput")
    ones128b_in = nc.dram_tensor("ones128b", [128, 128], BF16, kind="ExternalInput")
    ones128f_in = nc.dram_tensor("ones128f", [128, 128], F32, kind="ExternalInput")
    mlp_in = nc.dram_tensor("mlp_rhs", [66, 32], F32, kind="ExternalInput")
    w5_in = nc.dram_tensor("w5rep", [128, 32], F32, kind="ExternalInput")
    b5_in = nc.dram_tensor("b5rep", [128, 1], F32, kind="ExternalInput")
    out_dram = nc.dram_tensor("out", [BC, 1], F32, kind="ExternalOutput")

    with TileContext(nc) as tc:
        with (
            tc.tile_pool(name="const", bufs=1) as cp,
            tc.tile_pool(name="sb", bufs=3) as sb,
            tc.tile_pool(name="psz1", bufs=1, space="PSUM") as psz1,
            tc.tile_pool(name="pszB", bufs=1, space="PSUM") as pszB,
            tc.tile_pool(name="psg", bufs=3, space="PSUM") as psg,
            tc.tile_pool(name="pss", bufs=1, space="PSUM") as pss,
        ):
            # ---- constants ----
            w1r = cp.tile([128, 3, 256], BF16, tag="w1r")
            nc.sync.dma_start(out=w1r[:], in_=w1_in[:])
            w2t = cp.tile([128, 3, 2, 128], BF16, tag="w2t")
            nc.sync.dma_start(out=w2t[:], in_=w2_in[:])
            w3t = cp.tile([128, 3, 64], BF16, tag="w3t")
            nc.sync.dma_start(out=w3t[:], in_=w3_in[:])
            biasrows = cp.tile([1, 384], BF16, tag="biasrows")
            nc.sync.dma_start(out=biasrows[:], in_=br_in[:])
            onescol = cp.tile([1, 128], BF16, tag="onescol")
            nc.sync.dma_start(out=onescol[:], in_=oc_in[:])
            bcols = cp.tile([128, 8], F32, tag="bcols")
            nc.sync.dma_start(out=bcols[:], in_=bc_in[:])
            ones128b = cp.tile([128, 128], BF16, tag="ones128b")
            nc.sync.dma_start(out=ones128b[:], in_=ones128b_in[:])
            ones128f = cp.tile([128, 128], F32, tag="ones128f")
            nc.sync.dma_start(out=ones128f[:], in_=ones128f_in[:])
            mlp_rhs = cp.tile([66, 32], F32, tag="mlp_rhs")
            nc.sync.dma_start(out=mlp_rhs[:], in_=mlp_in[:])
            w5rep = cp.tile([128, 32], F32, tag="w5rep")
            nc.sync.dma_start(out=w5rep[:], in_=w5_in[:])
            b5rep = cp.tile([128, 1], F32, tag="b5rep")
            nc.sync.dma_start(out=b5rep[:], in_=b5_in[:])

            for g in range(NGROUPS):
                t0 = g * G
                # ---- inputs for this group ----
                x1T = sb.tile([128, G, 128], BF16, tag="x1T")
                nc.sync.dma_start(out=x1T[:], in_=treesT_in[:, t0:t0 + G, :])
                P = sb.tile([128, G, 384], BF16, tag="P")
                nc.sync.dma_start(out=P[:], in_=p_in[:, t0:t0 + G, :])

                # ---- layer 1 gathers: g1[c, j] = x1[c, idx[j]] ----
                g1 = sb.tile([128, 3, G, 128], BF16, tag="g1")
                for t in range(G):
                    gp = psg.tile([128, 384], F32, tag="g", name=f"g1p{t}")
                    nc.tensor.matmul(gp[:], x1T[:, t, :], P[:, t, :],
                                     start=True, stop=True)
                    nc.scalar.activation(
                        _ap(g1[:], [[G * 128, 3], [1, 128]], t * 128),
                        gp[:], mybir.ActivationFunctionType.Copy,
                        bias=0.0, scale=1.0)

                # ---- layer 1 convs (reversed): z1T[m, o] per tree ----
                z1T = psz1.tile([128, G, 256], F32, tag="z1")
                for t in range(G):
                    for k in range(3):
                        nc.tensor.matmul(
                            z1T[:, t, :], g1[:, k, t, :], w1r[:, k, :],
                            start=(t % 2 == 0 and k == 0), stop=(k == 2),
                            skip_group_check=True)

                # LN1 mean: per-row sums -> fused -mean via ones MM
                s1 = sb.tile([128, G], F32, tag="s1")
                nc.vector.tensor_reduce(
                    s1[:], _ap(z1T[:], [[256, G], [1, 256]]),
                    axis=mybir.AxisListType.X, op=mybir.AluOpType.add)
                s1n = sb.tile([128, G], BF16, tag="s1n")
                with nc.allow_low_precision(reason="mean shift in bf16"):
                    nc.vector.tensor_scalar(
                        s1n[:], s1[:], -1.0 / K1, bcols[:, 5:6],
                        mybir.AluOpType.mult, mybir.AluOpType.add)
                # bias (rows 0..126 only) + mean shift (all 128 rows)
                z1Tr = z1T[0:127, :, :]
                for h in range(2):
                    nc.tensor.matmul(
                        bass.AP(tensor=z1Tr.tensor,
                                offset=z1Tr.offset + h * 512,
                                ap=[z1Tr.ap[0], [256, 2], [1, 256]]),
                        onescol[:, 0:127],
                        _ap(biasrows[:], [[0, 2], [1, 256]]),
                        start=False, stop=False, skip_group_check=True)
                    nc.tensor.matmul(
                        _ap(z1T[:], [[256, 2], [1, 256]], h * 512),
                        ones128b[:],
                        bass.AP(tensor=s1n.tensor,
                                offset=s1n[:].offset + h * 2,
                                ap=[s1n[:].ap[0], [1, 2], [0, 256]]),
                        start=False, stop=True, skip_group_check=True)

                # X2 node-major [row, t, c]: row r = node r+1 for r<127;
                # row 127 = node 0 = leaky(-mean) (PSUM pad row, no bias).
                x2 = sb.tile([128, G, 256], BF16, tag="x2")
                nc.scalar.activation(
                    x2[:], z1T[:],
                    mybir.ActivationFunctionType.Prelu,
                    bias=0.0, scale=1.0, alpha=_ALPHA)

                # ---- layer 2 gathers (stationary = x2 chunks) ----
                g2 = sb.tile([128, 2, 3, G, 128], BF16, tag="g2")
                for t in range(G):
                    for j in range(2):
                        gp = psg.tile([128, 384], F32, tag="g",
                                      name=f"g2p{t}{j}")
                        nc.tensor.matmul(
                            gp[:], x2[:, t, j * 128:(j + 1) * 128],
                            P[:, t, :], start=True, stop=True)
                        nc.scalar.activation(
                            _ap(g2[:], [[G * 128, 3], [1, 128]],
                                j * 3 * G * 128 + t * 128),
                            gp[:], mybir.ActivationFunctionType.Copy,
                            bias=0.0, scale=1.0)

                # ---- layer 2 convs (reversed): z2T[m, o] ----
                z2T = pszB.tile([128, G, 128], F32, tag="z2")
                for t in range(G):
                    for j in range(2):
                        for k in range(3):
                            nc.tensor.matmul(
                                z2T[:, t, :], g2[:, j, k, t, :],
                                w2t[:, k, j, :],
                                start=(t == 0 and j == 0 and k == 0),
                                stop=(k == 2 and j == 1),
                                skip_group_check=True)

                # LN2 mean
                s2 = sb.tile([128, G], F32, tag="s2")
                nc.vector.tensor_reduce(
                    s2[:], _ap(z2T[:], [[128, G], [1, 128]]),
                    axis=mybir.AxisListType.X, op=mybir.AluOpType.add)
                s2n = sb.tile([128, G], BF16, tag="s2n")
                with nc.allow_low_precision(reason="mean shift in bf16"):
                    nc.vector.tensor_scalar(
                        s2n[:], s2[:], -1.0 / K2, bcols[:, 6:7],
                        mybir.AluOpType.mult, mybir.AluOpType.add)
                z2Tr = z2T[0:127, :, :]
                nc.tensor.matmul(
                    bass.AP(tensor=z2Tr.tensor, offset=z2Tr.offset,
                            ap=[z2Tr.ap[0], [128, G], [1, 128]]),
                    onescol[:, 0:127],
                    _ap(biasrows[:], [[0, G], [1, 128]], 256),
                    start=False, stop=False, skip_group_check=True)
                nc.tensor.matmul(
                    _ap(z2T[:], [[128, G], [1, 128]]),
                    ones128b[:],
                    _ap(s2n[:], [[1, G], [0, 128]]),
                    start=False, stop=True, skip_group_check=True)

                # X3 node-major, same row permutation as X2
                x3 = sb.tile([128, G, 128], BF16, tag="x3")
                nc.scalar.activation(
                    x3[:], z2T[:],
                    mybir.ActivationFunctionType.Prelu,
                    bias=0.0, scale=1.0, alpha=_ALPHA)

                # ---- layer 3: gather, channel-major conv, LN, max-pool ----
                g3 = sb.tile([128, 3, G, 128], BF16, tag="g3")
                for t in range(G):
                    gp = psg.tile([128, 384], F32, tag="g", name=f"g3p{t}")
                    nc.tensor.matmul(gp[:], x3[:, t, :], P[:, t, :],
                                     start=True, stop=True)
                    nc.scalar.activation(
                        _ap(g3[:], [[G * 128, 3], [1, 128]], t * 128),
                        gp[:], mybir.ActivationFunctionType.Copy,
                        bias=0.0, scale=1.0)

                z3 = pszB.tile([64, G, 128], F32, tag="z3")
                z3_all = _ap(z3[:], [[128, G], [1, 128]])
                for k in range(3):
                    rhs = _ap(g3[:], [[128, G], [1, 128]], k * G * 128)
                    nc.tensor.matmul(z3_all, w3t[:, k, :], rhs,
                                     start=(k == 0), stop=(k == 2))

                # LN3 stats on z3+b3 (b3 via ACT bias / post-corrections)
                z3v = _ap(z3[:], [[128, G], [1, 127]])
                s3 = sb.tile([64, 2, G], F32, tag="s3")
                nc.vector.tensor_reduce(
                    s3[:, 0, :], z3v, axis=mybir.AxisListType.X,
                    op=mybir.AluOpType.add)
                nc.vector.tensor_scalar(
                    s3[:, 0, :], s3[:, 0, :], bcols[0:64, 4:5], None,
                    mybir.AluOpType.add)
                sq = sb.tile([64, G, 127], F32, tag="sq")
                nc.scalar.activation(
                    sq[:], z3v, mybir.ActivationFunctionType.Square,
                    bias=bcols[0:64, 3:4], scale=1.0)
                nc.vector.tensor_reduce(
                    s3[:, 1, :], sq[:], axis=mybir.AxisListType.X,
                    op=mybir.AluOpType.add)
                ps3 = pss.tile([128, 2, G], F32, tag="pss", name="ps3")
                nc.tensor.matmul(
                    ps3[:], ones128f[0:64, :],
                    bass.AP(tensor=s3.tensor, offset=s3[:].offset,
                            ap=[s3[:].ap[0], [1, 2 * G]]),
                    start=True, stop=True)
                # mean3 = S/K3; nm3 = -mean3; var = SS/(K3-1) - K3/(K3-1)*mean^2
                mean3 = sb.tile([128, G], F32, tag="mean3")
                nc.vector.tensor_scalar(
                    mean3[:], ps3[:, 0, :], 1.0 / K3, None, mybir.AluOpType.mult)
                nm3 = sb.tile([128, G], F32, tag="nm3")
                nc.vector.tensor_scalar(
                    nm3[:], mean3[:], -1.0, None, mybir.AluOpType.mult)
                m3sq = sb.tile([128, G], F32, tag="m3sq")
                nc.vector.tensor_tensor(
                    m3sq[:], mean3[:], mean3[:], mybir.AluOpType.mult)
                var3 = sb.tile([128, G], F32, tag="var3")
                nc.vector.tensor_scalar(
                    var3[:], m3sq[:], -float(K3) / (K3 - 1), None,
                    mybir.AluOpType.mult)
                ssn = sb.tile([128, G], F32, tag="ssn")
                nc.vector.tensor_scalar(
                    ssn[:], ps3[:, 1, :], 1.0 / (K3 - 1), None,
                    mybir.AluOpType.mult)
                nc.vector.tensor_tensor(
                    var3[:], var3[:], ssn[:], mybir.AluOpType.add)
                std3 = sb.tile([128, G], F32, tag="std3")
                nc.scalar.activation(
                    std3[:], var3[:], mybir.ActivationFunctionType.Sqrt,
                    bias=0.0, scale=1.0)
                nc.vector.tensor_scalar(
                    std3[:], std3[:], 1e-5, None, mybir.AluOpType.add)
                sinv3 = sb.tile([128, G], F32, tag="sinv3")
                nc.vector.reciprocal(sinv3[:], std3[:])

                # pooled = sinv3 * (max(max_m z3 + b3, 0) - mean3)
                pr = sb.tile([64, G], F32, tag="pr")
                nc.vector.tensor_reduce(
                    pr[:], z3v, axis=mybir.AxisListType.X,
                    op=mybir.AluOpType.max)
                paug = sb.tile([66, G], F32, tag="paug")
                nc.vector.memset(paug[64:66, :], 1.0)
                r1 = sb.tile([64, G], F32, tag="r1")
                nc.vector.tensor_scalar(
                    r1[:], pr[:], bcols[0:64, 3:4], 0.0,
                    mybir.AluOpType.add, mybir.AluOpType.max)
                r2 = sb.tile([64, G], F32, tag="r2")
                nc.vector.tensor_tensor(
                    r2[:], r1[:], nm3[0:64, :], mybir.AluOpType.add)
                nc.vector.tensor_tensor(
                    paug[0:64, :], r2[:], sinv3[0:64, :], mybir.AluOpType.mult)

                # h = leaky(W4 @ pooled + b4); out = h @ W5.T + b5
                ph = pss.tile([G, 32], F32, tag="pss", name="ph")
                nc.tensor.matmul(ph[:], paug[:, :], mlp_rhs[0:66, :],
                                 start=True, stop=True)
                h = sb.tile([G, 32], F32, tag="h")
                nc.scalar.activation(
                    h[:], ph[:], mybir.ActivationFunctionType.Prelu,
                    bias=0.0, scale=1.0, alpha=_ALPHA)
                prod = sb.tile([G, 32], F32, tag="prod")
                nc.vector.tensor_tensor(
                    prod[:], h[:], w5rep[0:G, :], mybir.AluOpType.mult)
                ov = sb.tile([G, 1], F32, tag="ov")
                nc.vector.tensor_reduce(
                    ov[:], prod[:], axis=mybir.AxisListType.X,
                    op=mybir.AluOpType.add)
                nc.vector.tensor_scalar(
                    ov[:], ov[:], b5rep[0:G, :], None, mybir.AluOpType.add)
                nc.sync.dma_start(out=out_dram[t0:t0 + G, :], in_=ov[:])

    _bass_rust.generate_event_semaphores(nc)
    nc.finalize()
    return nc


_NC_CACHE = None


def _get_nc():
    global _NC_CACHE
    if _NC_CACHE is None:
        _NC_CACHE = build_nc()
    return _NC_CACHE


def _prep_idx_flat(indexes: np.ndarray) -> np.ndarray:
    """indexes [B, 381] -> k-major [B, 384] int32 with pads = -1."""
    b = indexes.shape[0]
    idxk = np.full((b, 3, 128), -1, np.int32)
    tri = indexes.reshape(b, 127, 3).astype(np.int32)
    idxk[:, :, :127] = tri.transpose(0, 2, 1)
    return idxk.reshape(b, 384)


def kernel(trees, W1, b1, W2, b2, W3, b3, W4, b4, W5, b5, indexes):
    trees = np.asarray(trees, dtype=np.float32)
    indexes = np.asarray(indexes).astype(np.int64)
    W1 = np.asarray(W1, dtype=np.float32)
    W2 = np.asarray(W2, dtype=np.float32)
    W3 = np.asarray(W3, dtype=np.float32)
    W4 = np.asarray(W4, dtype=np.float32)
    W5 = np.asarray(W5, dtype=np.float32)
    b1 = np.asarray(b1, dtype=np.float32)
    b2 = np.asarray(b2, dtype=np.float32)
    b3 = np.asarray(b3, dtype=np.float32)
    b4 = np.asarray(b4, dtype=np.float32)
    b5 = np.asarray(b5, dtype=np.float32)

    nc = _get_nc()

    bf = ml_dtypes.bfloat16
    # replicated weight prep
    # w1r[c, k, o] = W1[o, c, k]
    w1r = np.ascontiguousarray(W1.transpose(1, 2, 0)).astype(bf)
    # w2t[p, k, j, o] = W2[o, j*128+p, k]
    w2t = np.ascontiguousarray(
        W2.reshape(128, 2, 128, 3).transpose(2, 3, 1, 0)).astype(bf)
    # w3t[c, k, o] = W3[o, c, k]
    w3t = np.ascontiguousarray(W3.transpose(1, 2, 0)).astype(bf)
    biasrows = np.zeros((1, 384), np.float32)
    biasrows[0, :256] = b1
    biasrows[0, 256:] = b2
    biasrows = biasrows.astype(bf)
    onescol = np.ones((1, 128), bf)
    # bias columns + scalar mean-bias corrections
    bcols = np.zeros((128, 8), np.float32)
    bcols[:64, 3] = b3
    bcols[:64, 4] = 127.0 * b3
    bcols[:, 5] = -127.0 * float(b1.sum()) / (K1 * 128)  # L1 shift mean-bias
    bcols[:, 6] = -127.0 * float(b2.sum()) / (K2 * 128)  # L2 shift mean-bias
    ones128b = np.ones((128, 128), bf)
    ones128f = np.ones((128, 128), np.float32)
    mlp_rhs = np.zeros((66, 32), np.float32)
    mlp_rhs[:64] = W4.T
    mlp_rhs[64] = b4 * 0.5
    mlp_rhs[65] = b4 * 0.5
    w5rep = np.tile(W5.reshape(1, 32), (128, 1)).astype(np.float32)
    b5rep = np.full((128, 1), b5[0], np.float32)

    idxk = _prep_idx_flat(indexes)  # [B, 384] int32, pads -1
    # node -> row permutation: node n -> row n-1, node 0 -> row 127.
    # P[b, r, j] = (adj[b, j] == r); pads (-1) give zero columns.
    adj = np.where(idxk < 0, -1, np.where(idxk == 0, 127, idxk - 1))
    onehot = (adj[:, None, :] == np.arange(128, dtype=np.int32)[None, :, None])
    onehot = onehot.astype(bf)  # [B, 128, 384]
    nodeperm = np.concatenate([np.arange(1, 128), [0]])

    in_maps = []
    for c in range(N_CORES):
        lo, hi = c * BC, (c + 1) * BC
        treesT = np.ascontiguousarray(
            trees[lo:hi][:, :, nodeperm].transpose(2, 0, 1)).astype(bf)
        ponehot = np.ascontiguousarray(
            onehot[lo:hi].transpose(1, 0, 2))  # [128, BC, 384]
        in_maps.append({
            "treesT": treesT,
            "ponehot": ponehot,
            "w1r": w1r, "w2t": w2t, "w3t": w3t,
            "biasrows": biasrows, "onescol": onescol, "bcols": bcols,
            "ones128b": ones128b, "ones128f": ones128f,
            "mlp_rhs": mlp_rhs, "w5rep": w5rep, "b5rep": b5rep,
        })

    global _LAST_IN_MAPS
    _LAST_IN_MAPS = in_maps
    res = run_bass_kernel_spmd(nc, in_maps, list(range(N_CORES)))
    out = np.concatenate([res.results[c]["out"] for c in range(N_CORES)], axis=0)
    return out.astype(np.float32)


_LAST_IN_MAPS = None
